# revision 7
# baseline (speedup 1.0000x reference)
"""Dynamic per-sample CNN (nn_ConvFunc) Trainium2 Bass kernel.

Reference computation (per sample b):
  cnn_inp = proj_w @ cat(lhs, rhs) + proj_b          # 1x1 conv, [128, 32, 32]
  out     = conv3x3(cnn_inp, W_b) + bias_b           # W_b, bias_b unpacked from question_rep[b]

Sharding: pure data parallel, 8 samples per NeuronCore (batch 64 / 8 cores).

Per-core device kernel, per sample:
  - proj: per 512-col half of the 32x32 pixel space, 2 accumulating bf16
    matmuls (lhs-channels, rhs-channels) into one PSUM bank; ACT evicts PSUM
    (+proj_b per-partition bias) into the interior of a zero-bordered
    [128,34,34] padded SBUF tile;
  - conv: per half, 9 taps of accumulating bf16 matmuls; rhs = shifted 16x32
    window of the padded tile (strided AP); DVE evicts PSUM (+cnn bias) to
    bf16 SBUF, one store DMA per sample (halves for the last sample so the
    final store issues early).

All matmul operands are bf16 (rounded on host): the PE streams 1 col/cycle
either way, but bf16 halves HBM traffic and triggers FWL so LDWEIGHTS hides
behind the previous matmul. lhs/rhs are packed host-side into one DRAM
tensor per sample (1 queue op); input loads are split across both HWDGE
engines (sync + scalar) to cut queue-op serialization at the start. A run
of short dummy matmuls on a zeroed scratch tile bridges the PE through the
DMA ramp so the HAM clock gate lifts before real work begins.
"""

import numpy as np
import ml_dtypes

import concourse.bass as bass
import concourse.mybir as mybir
from concourse import bacc
from concourse.tile import TileContext
from concourse.bass_utils import run_bass_kernel_spmd

# Problem shapes (hardcoded per contract)
B = 64
DIM = 128
H = W = 32
K = 3
KK = K * K
HW = H * W             # 1024
WDIM = DIM * DIM * KK  # 147456
NCORES = 8
SPC = B // NCORES      # samples per core
HP, WP = H + 2, W + 2  # padded 34x34
HALF = HW // 2         # 512 columns per PSUM bank
HROWS = H // 2         # 16 output rows per half
NXP = 4                # persistent padded tiles (cycled s % NXP)
N_WARM = 26            # dummy warmup matmuls (N=128 each)
QUART = HALF // 2      # final-store split size

FP = mybir.dt.float32
BF = mybir.dt.bfloat16
BF_NP = ml_dtypes.bfloat16

_BUILT = {}


def build_nc():
    nc = bacc.Bacc("TRN2", target_bir_lowering=False, debug=False,
                   num_devices=NCORES)

    # xc packs [xl_h0 | xr_h0 | xl_h1 | xr_h1] per sample (one load DMA)
    xc = nc.declare_dram_parameter("xc", [SPC, DIM, 4, HALF], BF, isOutput=False)
    qw = nc.declare_dram_parameter("qw", [SPC, DIM, KK * DIM], BF, isOutput=False)
    pw = nc.declare_dram_parameter("pw", [DIM, 2 * DIM], BF, isOutput=False)
    bia = nc.declare_dram_parameter("bia", [DIM, SPC + 1], FP, isOutput=False)
    out = nc.declare_dram_parameter("out", [SPC, DIM, HW], BF, isOutput=True)

    with TileContext(nc) as tc:
        with (
            tc.tile_pool(name="const", bufs=1) as cpool,
            tc.tile_pool(name="wpool", bufs=4) as wpool,
            tc.tile_pool(name="xpool", bufs=4) as xpool,
            tc.tile_pool(name="opool", bufs=4) as opool,
            tc.tile_pool(name="pp_pool", bufs=3, space="PSUM") as pp_pool,
            tc.tile_pool(name="pc_pool", bufs=5, space="PSUM") as pc_pool,
        ):
            # --- warmup: short dummy matmuls keep the PE busy through the
            # DMA ramp so the HAM clock gate lifts before real work
            dummy = cpool.tile([DIM, DIM], BF)
            nc.vector.memset(dummy[:], 0.0)
            pdt = pc_pool.tile([DIM, HALF], FP, tag="pc")
            for _ in range(N_WARM):
                nc.tensor.matmul(pdt[:, 0:DIM], lhsT=dummy[:], rhs=dummy[:],
                                 start=True, stop=True)

            # --- constants: proj weights (bf16, sync) + biases (fp32, scalar)
            pw_sb = cpool.tile([DIM, 2 * DIM], BF)
            nc.sync.dma_start(out=pw_sb[:], in_=pw[:])
            bia_sb = cpool.tile([DIM, SPC + 1], FP)
            nc.scalar.dma_start(out=bia_sb[:], in_=bia[:])
            pw0 = pw_sb[:, 0:DIM]
            pw1 = pw_sb[:, DIM:2 * DIM]

            def qb_ap(s):
                return bia_sb[:, s:s + 1]

            pb_ap = bia_sb[:, SPC:SPC + 1]

            # --- persistent padded tiles: borders zeroed once, interiors
            # rewritten per sample
            xp_tiles = []
            for i in range(NXP):
                xp = cpool.tile([DIM, HP, WP], BF, tag=f"xp{i}")
                nc.vector.memset(xp[:, 0:1, :], 0.0)
                nc.vector.memset(xp[:, HP - 1:HP, :], 0.0)
                nc.vector.memset(xp[:, 1:HP - 1, 0:1], 0.0)
                nc.vector.memset(xp[:, 1:HP - 1, WP - 1:WP], 0.0)
                xp_tiles.append(xp)

            def load_x(s):
                xc_sb = xpool.tile([DIM, 4, HALF], BF, tag="xc")
                if s == 0:
                    # sample 0 is latency-critical: small pieces, in the
                    # order the first matmuls consume them
                    nc.sync.dma_start(out=xc_sb[:, 0:1], in_=xc[s, :, 0:1])
                    nc.sync.dma_start(out=xc_sb[:, 1:2], in_=xc[s, :, 1:2])
                    nc.sync.dma_start(out=xc_sb[:, 2:4], in_=xc[s, :, 2:4])
                else:
                    nc.sync.dma_start(out=xc_sb[:], in_=xc[s])
                return xc_sb

            def load_w(s):
                w_sb = wpool.tile([DIM, KK, DIM], BF, tag="w")
                nc.scalar.dma_start(out=w_sb[:], in_=qw[s])
                return w_sb

            def proj(s, xc_sb):
                xp = xp_tiles[s % NXP]
                for h in range(2):
                    ppt = pp_pool.tile([DIM, HALF], FP, tag="pp")
                    nc.tensor.matmul(ppt[:], lhsT=pw0, rhs=xc_sb[:, 2 * h],
                                     start=True, stop=False)
                    nc.tensor.matmul(ppt[:], lhsT=pw1, rhs=xc_sb[:, 2 * h + 1],
                                     start=False, stop=True)
                    nc.scalar.activation(
                        xp[:, 1 + HROWS * h:1 + HROWS * (h + 1), 1:1 + W],
                        ppt[:].rearrange("p (a b) -> p a b", b=W),
                        mybir.ActivationFunctionType.Identity,
                        bias=pb_ap,
                    )
                return xp

            def conv(s, xp, w_sb):
                o_sb = opool.tile([DIM, HW], BF, tag="o")
                for h in range(2):
                    pct = pc_pool.tile([DIM, HALF], FP, tag="pc")
                    for t in range(KK):
                        kh, kw = divmod(t, K)
                        nc.tensor.matmul(
                            pct[:],
                            lhsT=w_sb[:, t, :],
                            rhs=xp[:, HROWS * h + kh:HROWS * (h + 1) + kh,
                                   kw:kw + W],
                            start=(t == 0), stop=(t == KK - 1))
                    last = s == SPC - 1
                    if last and h == 1:
                        # final half: evict + store quarter-wise on BOTH
                        # engine pairs in parallel to minimize the tail
                        nc.vector.tensor_scalar_add(
                            o_sb[:, HALF:HALF + QUART],
                            pct[:, 0:QUART], qb_ap(s))
                        nc.scalar.activation(
                            o_sb[:, HALF + QUART:HW],
                            pct[:, QUART:HALF],
                            mybir.ActivationFunctionType.Identity,
                            bias=qb_ap(s))
                        nc.sync.dma_start(
                            out=out[s, :, HALF:HALF + QUART],
                            in_=o_sb[:, HALF:HALF + QUART])
                        nc.scalar.dma_start(
                            out=out[s, :, HALF + QUART:HW],
                            in_=o_sb[:, HALF + QUART:HW])
                        continue
                    nc.vector.tensor_scalar_add(
                        o_sb[:, h * HALF:(h + 1) * HALF], pct[:], qb_ap(s))
                    if last:
                        # store h0 as soon as it's ready
                        nc.sync.dma_start(
                            out=out[s, :, h * HALF:(h + 1) * HALF],
                            in_=o_sb[:, h * HALF:(h + 1) * HALF])
                if s != SPC - 1:
                    nc.sync.dma_start(out=out[s], in_=o_sb[:])

            # software pipeline: proj(s) ahead of conv(s-1) keeps PE dense;
            # w(s) before proj(s) so its queue op precedes the ACT evictions
            # and table load on the scalar queue
            xc0 = load_x(0)
            w0 = load_w(0)
            prev = None
            for s in range(SPC):
                xc_sb = xc0 if s == 0 else load_x(s)
                w_sb = w0 if s == 0 else load_w(s)
                xp = proj(s, xc_sb)
                if prev is not None:
                    conv(*prev)
                prev = (s, xp, w_sb)
            conv(*prev)

    nc.compile()
    return nc


def _prep(question_rep, lhs_rep, rhs_rep, proj_w, proj_b):
    """Host-side shard + layout prep (cheap reshapes/casts only)."""
    qr = np.ascontiguousarray(question_rep, dtype=np.float32)
    # conv weights: [B, o, i, kh, kw] -> [B, i, (kh kw), o] so each tap is a
    # ready lhsT [i, o] block and the per-sample weight DMA is contiguous
    qw = qr[:, :WDIM].reshape(B, DIM, DIM, K, K).transpose(0, 2, 3, 4, 1)
    qw = np.ascontiguousarray(qw).astype(BF_NP).reshape(B, DIM, KK * DIM)
    qb = np.ascontiguousarray(qr[:, WDIM:])             # [B, 128] fp32
    xl = np.asarray(lhs_rep, dtype=np.float32).reshape(B, DIM, 2, HALF)
    xr = np.asarray(rhs_rep, dtype=np.float32).reshape(B, DIM, 2, HALF)
    # pack [xl_h0 | xr_h0 | xl_h1 | xr_h1] -> [B, DIM, 4, HALF]
    xcm = np.stack([xl[:, :, 0], xr[:, :, 0], xl[:, :, 1], xr[:, :, 1]],
                   axis=2).astype(BF_NP)
    pwt = np.asarray(proj_w, dtype=np.float32).T.astype(BF_NP)  # [256, 128]
    pwm = np.ascontiguousarray(
        np.concatenate([pwt[:DIM], pwt[DIM:]], axis=1))  # [128, 256]
    pb = np.asarray(proj_b, dtype=np.float32).reshape(DIM, 1)

    in_maps = []
    for c in range(NCORES):
        sl = slice(c * SPC, (c + 1) * SPC)
        biam = np.ascontiguousarray(
            np.concatenate([qb[sl].T, pb], axis=1), dtype=np.float32)
        in_maps.append({
            "qw": np.ascontiguousarray(qw[sl]),
            "xc": np.ascontiguousarray(xcm[sl]),
            "pw": pwm,
            "bia": biam,
        })
    return in_maps


def kernel(question_rep, lhs_rep, rhs_rep, proj_w, proj_b, _run_kwargs=None):
    if "nc" not in _BUILT:
        _BUILT["nc"] = build_nc()
    nc = _BUILT["nc"]
    in_maps = _prep(question_rep, lhs_rep, rhs_rep, proj_w, proj_b)
    res = run_bass_kernel_spmd(nc, in_maps, core_ids=list(range(NCORES)),
                               **(_run_kwargs or {}))
    out = np.concatenate(
        [np.asarray(res.results[c]["out"], dtype=np.float32)
         for c in range(NCORES)], axis=0)
    if _run_kwargs is not None:
        _BUILT["last_result"] = res
    return out.reshape(B, DIM, H, W)


if __name__ == "__main__":
    rng = np.random.default_rng(0)
    inputs = {
        "question_rep": rng.standard_normal((B, WDIM + DIM), dtype=np.float32) * 0.05,
        "lhs_rep": rng.standard_normal((B, DIM, H, W), dtype=np.float32),
        "rhs_rep": rng.standard_normal((B, DIM, H, W), dtype=np.float32),
        "proj_w": rng.standard_normal((DIM, 2 * DIM), dtype=np.float32),
        "proj_b": rng.standard_normal((DIM,), dtype=np.float32) * 0.01,
    }
    out = kernel(**inputs)
    print("ran, out shape:", out.shape)


# revision 10
# speedup vs baseline: 1.0759x; 1.0759x over previous
"""Dynamic per-sample CNN (nn_ConvFunc) Trainium2 Bass kernel.

Reference computation (per sample b):
  cnn_inp = proj_w @ cat(lhs, rhs) + proj_b          # 1x1 conv, [128, 32, 32]
  out     = conv3x3(cnn_inp, W_b) + bias_b           # W_b, bias_b unpacked from question_rep[b]

Sharding: pure data parallel, 8 samples per NeuronCore (batch 64 / 8 cores).

Per-core device kernel, per sample:
  - proj: per 512-col half of the 32x32 pixel space, 2 accumulating bf16
    matmuls (lhs-channels, rhs-channels) into one PSUM bank; ACT evicts PSUM
    (+proj_b per-partition bias) into the interior of a zero-bordered
    [128,34,34] padded SBUF tile;
  - conv: per half, 9 taps of accumulating bf16 matmuls; rhs = shifted 16x32
    window of the padded tile (strided AP); DVE evicts PSUM (+cnn bias) to
    bf16 SBUF, one store DMA per sample (halves for the last sample so the
    final store issues early).

All matmul operands are bf16 (rounded on host): the PE streams 1 col/cycle
either way, but bf16 halves HBM traffic and triggers FWL so LDWEIGHTS hides
behind the previous matmul. lhs/rhs are packed host-side into one DRAM
tensor per sample (1 queue op); input loads are split across both HWDGE
engines (sync + scalar) to cut queue-op serialization at the start. A run
of short dummy matmuls on a zeroed scratch tile bridges the PE through the
DMA ramp so the HAM clock gate lifts before real work begins.
"""

import numpy as np
import ml_dtypes

import concourse.bass as bass
import concourse.mybir as mybir
from concourse import bacc
from concourse.tile import TileContext
from concourse.bass_utils import run_bass_kernel_spmd

# Problem shapes (hardcoded per contract)
B = 64
DIM = 128
H = W = 32
K = 3
KK = K * K
HW = H * W             # 1024
WDIM = DIM * DIM * KK  # 147456
NCORES = 8
SPC = B // NCORES      # samples per core
HP, WP = H + 2, W + 2  # padded 34x34
HALF = HW // 2         # 512 columns per PSUM bank
HROWS = H // 2         # 16 output rows per half
NXP = 4                # persistent padded tiles (cycled s % NXP)
N_WARM = 26            # dummy warmup matmuls (N=128 each)
QUART = HALF // 2      # final-store split size

FP = mybir.dt.float32
BF = mybir.dt.bfloat16
BF_NP = ml_dtypes.bfloat16

_BUILT = {}


def build_nc():
    nc = bacc.Bacc("TRN2", target_bir_lowering=False, debug=False,
                   num_devices=NCORES)

    # xc packs [xl_h0 | xr_h0 | xl_h1 | xr_h1] per sample (one load DMA)
    xc = nc.declare_dram_parameter("xc", [SPC, DIM, 4, HALF], BF, isOutput=False)
    qw = nc.declare_dram_parameter("qw", [SPC, DIM, KK * DIM], BF, isOutput=False)
    pw = nc.declare_dram_parameter("pw", [DIM, 2 * DIM], BF, isOutput=False)
    bia = nc.declare_dram_parameter("bia", [DIM, SPC + 1], FP, isOutput=False)
    out = nc.declare_dram_parameter("out", [SPC, DIM, HW], BF, isOutput=True)

    with TileContext(nc) as tc:
        with (
            tc.tile_pool(name="const", bufs=1) as cpool,
            tc.tile_pool(name="wpool", bufs=4) as wpool,
            tc.tile_pool(name="xpool", bufs=4) as xpool,
            tc.tile_pool(name="opool", bufs=4) as opool,
            tc.tile_pool(name="pp_pool", bufs=3, space="PSUM") as pp_pool,
            tc.tile_pool(name="pc_pool", bufs=5, space="PSUM") as pc_pool,
        ):
            # --- warmup: short dummy matmuls keep the PE busy through the
            # DMA ramp so the HAM clock gate lifts before real work
            dummy = cpool.tile([DIM, DIM], BF)
            nc.vector.memset(dummy[:], 0.0)
            pdt = pc_pool.tile([DIM, HALF], FP, tag="pc")
            for _ in range(N_WARM):
                nc.tensor.matmul(pdt[:, 0:DIM], lhsT=dummy[:], rhs=dummy[:],
                                 start=True, stop=True)

            # --- constants: proj weights (bf16, sync) + biases (fp32, scalar)
            pw_sb = cpool.tile([DIM, 2 * DIM], BF)
            nc.sync.dma_start(out=pw_sb[:], in_=pw[:])
            bia_sb = cpool.tile([DIM, SPC + 1], FP)
            nc.scalar.dma_start(out=bia_sb[:], in_=bia[:])
            pw0 = pw_sb[:, 0:DIM]
            pw1 = pw_sb[:, DIM:2 * DIM]

            def qb_ap(s):
                return bia_sb[:, s:s + 1]

            pb_ap = bia_sb[:, SPC:SPC + 1]

            # --- persistent padded tiles: borders zeroed once, interiors
            # rewritten per sample
            xp_tiles = []
            for i in range(NXP):
                xp = cpool.tile([DIM, HP, WP], BF, tag=f"xp{i}")
                nc.vector.memset(xp[:, 0:1, :], 0.0)
                nc.vector.memset(xp[:, HP - 1:HP, :], 0.0)
                nc.vector.memset(xp[:, 1:HP - 1, 0:1], 0.0)
                nc.vector.memset(xp[:, 1:HP - 1, WP - 1:WP], 0.0)
                xp_tiles.append(xp)

            def load_x(s):
                xc_sb = xpool.tile([DIM, 4, HALF], BF, tag="xc")
                if s == 0:
                    # sample 0 is latency-critical: halves, in the order the
                    # first matmuls consume them
                    nc.sync.dma_start(out=xc_sb[:, 0:2], in_=xc[s, :, 0:2])
                    nc.sync.dma_start(out=xc_sb[:, 2:4], in_=xc[s, :, 2:4])
                else:
                    nc.sync.dma_start(out=xc_sb[:], in_=xc[s])
                return xc_sb

            def load_w(s):
                w_sb = wpool.tile([DIM, KK, DIM], BF, tag="w")
                nc.scalar.dma_start(out=w_sb[:], in_=qw[s])
                return w_sb

            def warm(n):
                for _ in range(n):
                    nc.tensor.matmul(pdt[:, 0:DIM], lhsT=dummy[:], rhs=dummy[:],
                                     start=True, stop=True)

            def proj(s, xc_sb):
                xp = xp_tiles[s % NXP]
                for h in range(2):
                    ppt = pp_pool.tile([DIM, HALF], FP, tag="pp")
                    nc.tensor.matmul(ppt[:], lhsT=pw0, rhs=xc_sb[:, 2 * h],
                                     start=True, stop=False)
                    nc.tensor.matmul(ppt[:], lhsT=pw1, rhs=xc_sb[:, 2 * h + 1],
                                     start=False, stop=True)
                    # DVE eviction (not ACT) keeps the ACT table load off the
                    # scalar queue head, so qw loads issue early
                    nc.vector.tensor_scalar_add(
                        xp[:, 1 + HROWS * h:1 + HROWS * (h + 1), 1:1 + W],
                        ppt[:].rearrange("p (a b) -> p a b", b=W),
                        pb_ap)
                return xp

            def conv(s, xp, w_sb):
                o_sb = opool.tile([DIM, HW], BF, tag="o")
                for h in range(2):
                    pct = pc_pool.tile([DIM, HALF], FP, tag="pc")
                    for t in range(KK):
                        kh, kw = divmod(t, K)
                        nc.tensor.matmul(
                            pct[:],
                            lhsT=w_sb[:, t, :],
                            rhs=xp[:, HROWS * h + kh:HROWS * (h + 1) + kh,
                                   kw:kw + W],
                            start=(t == 0), stop=(t == KK - 1))
                    last = s == SPC - 1
                    if last and h == 1:
                        # final half: evict + store quarter-wise on BOTH
                        # engine pairs in parallel to minimize the tail
                        nc.vector.tensor_scalar_add(
                            o_sb[:, HALF:HALF + QUART],
                            pct[:, 0:QUART], qb_ap(s))
                        nc.scalar.activation(
                            o_sb[:, HALF + QUART:HW],
                            pct[:, QUART:HALF],
                            mybir.ActivationFunctionType.Identity,
                            bias=qb_ap(s))
                        nc.sync.dma_start(
                            out=out[s, :, HALF:HALF + QUART],
                            in_=o_sb[:, HALF:HALF + QUART])
                        nc.scalar.dma_start(
                            out=out[s, :, HALF + QUART:HW],
                            in_=o_sb[:, HALF + QUART:HW])
                        continue
                    nc.vector.tensor_scalar_add(
                        o_sb[:, h * HALF:(h + 1) * HALF], pct[:], qb_ap(s))
                    if last:
                        # store h0 as soon as it's ready
                        nc.scalar.dma_start(
                            out=out[s, :, h * HALF:(h + 1) * HALF],
                            in_=o_sb[:, h * HALF:(h + 1) * HALF])
                if s != SPC - 1:
                    nc.scalar.dma_start(out=out[s], in_=o_sb[:])

            # software pipeline: proj(s) ahead of conv(s-1) keeps PE dense.
            # Samples 0-2 loads are prefetched upfront (queue ops are ~0.6us
            # each and loads land ~2.5us after queueing); interleaved dummy
            # matmuls plug early data-wait gaps so the HAM activity window
            # never resets during the ramp.
            xcs = {0: load_x(0), 1: load_x(1), 2: load_x(2)}
            ws = {0: load_w(0), 1: load_w(1), 2: load_w(2)}
            prev = None
            for s in range(SPC):
                xc_sb = xcs.pop(s) if s in xcs else load_x(s)
                w_sb = ws.pop(s) if s in ws else load_w(s)
                xp = proj(s, xc_sb)
                if s == 0:
                    warm(6)
                if prev is not None:
                    conv(*prev)
                if s == 1:
                    warm(4)
                prev = (s, xp, w_sb)
            conv(*prev)

    nc.compile()
    return nc


def _prep(question_rep, lhs_rep, rhs_rep, proj_w, proj_b):
    """Host-side shard + layout prep (cheap reshapes/casts only)."""
    qr = np.ascontiguousarray(question_rep, dtype=np.float32)
    # conv weights: [B, o, i, kh, kw] -> [B, i, (kh kw), o] so each tap is a
    # ready lhsT [i, o] block and the per-sample weight DMA is contiguous
    qw = qr[:, :WDIM].reshape(B, DIM, DIM, K, K).transpose(0, 2, 3, 4, 1)
    qw = np.ascontiguousarray(qw).astype(BF_NP).reshape(B, DIM, KK * DIM)
    qb = np.ascontiguousarray(qr[:, WDIM:])             # [B, 128] fp32
    xl = np.asarray(lhs_rep, dtype=np.float32).reshape(B, DIM, 2, HALF)
    xr = np.asarray(rhs_rep, dtype=np.float32).reshape(B, DIM, 2, HALF)
    # pack [xl_h0 | xr_h0 | xl_h1 | xr_h1] -> [B, DIM, 4, HALF]
    xcm = np.stack([xl[:, :, 0], xr[:, :, 0], xl[:, :, 1], xr[:, :, 1]],
                   axis=2).astype(BF_NP)
    pwt = np.asarray(proj_w, dtype=np.float32).T.astype(BF_NP)  # [256, 128]
    pwm = np.ascontiguousarray(
        np.concatenate([pwt[:DIM], pwt[DIM:]], axis=1))  # [128, 256]
    pb = np.asarray(proj_b, dtype=np.float32).reshape(DIM, 1)

    in_maps = []
    for c in range(NCORES):
        sl = slice(c * SPC, (c + 1) * SPC)
        biam = np.ascontiguousarray(
            np.concatenate([qb[sl].T, pb], axis=1), dtype=np.float32)
        in_maps.append({
            "qw": np.ascontiguousarray(qw[sl]),
            "xc": np.ascontiguousarray(xcm[sl]),
            "pw": pwm,
            "bia": biam,
        })
    return in_maps


def kernel(question_rep, lhs_rep, rhs_rep, proj_w, proj_b, _run_kwargs=None):
    if "nc" not in _BUILT:
        _BUILT["nc"] = build_nc()
    nc = _BUILT["nc"]
    in_maps = _prep(question_rep, lhs_rep, rhs_rep, proj_w, proj_b)
    res = run_bass_kernel_spmd(nc, in_maps, core_ids=list(range(NCORES)),
                               **(_run_kwargs or {}))
    out = np.concatenate(
        [np.asarray(res.results[c]["out"], dtype=np.float32)
         for c in range(NCORES)], axis=0)
    if _run_kwargs is not None:
        _BUILT["last_result"] = res
    return out.reshape(B, DIM, H, W)


if __name__ == "__main__":
    rng = np.random.default_rng(0)
    inputs = {
        "question_rep": rng.standard_normal((B, WDIM + DIM), dtype=np.float32) * 0.05,
        "lhs_rep": rng.standard_normal((B, DIM, H, W), dtype=np.float32),
        "rhs_rep": rng.standard_normal((B, DIM, H, W), dtype=np.float32),
        "proj_w": rng.standard_normal((DIM, 2 * DIM), dtype=np.float32),
        "proj_b": rng.standard_normal((DIM,), dtype=np.float32) * 0.01,
    }
    out = kernel(**inputs)
    print("ran, out shape:", out.shape)
